# revision 1
# baseline (speedup 1.0000x reference)
"""MoE per-sample expert conv3x3 (320->320, 64x64, B=16, 5 experts) on 8 trn2 cores.

Strategy: data-parallel over batch (2 samples/core). Host gathers each
sample's expert weights (weights[class_id]), transposes them to lhsT layout
[tap, CIN, COUT], and zero-pads features to 66x66 so the conv becomes 9
shifted matmuls accumulating in PSUM (no boundary handling on device).

float32r matmuls: full fp32 I/O, 11-bit mantissa multiplies, 4x the fp32
rate on the PE at N>=256.

CIN=320 splits into chunks (128,128,64). The 64-row remainder chunk would
leave half the PE array rows idle, so its kw=0/kw=1 tap pairs are packed
via tile_position row tiling: rows 0:64 of the array run tap (kh,0) while
rows 64:128 concurrently run tap (kh,1), whose weights are overlaid in the
upper partition half of the chunk-2 weight tile and whose features come
from a +1-shifted duplicate in the upper half of the chunk-2 feature tile.
The row-B results accumulate in a second PSUM bank per n-tile and are
folded in during the bias epilogue.

DMA: features on the SP HWDGE ring (nc.sync), weights/bias/outputs on the
ACT ring (nc.scalar) so the two input streams load in parallel at startup.
"""

import time

import numpy as np

import concourse.bass as bass
import concourse.mybir as mybir
import concourse.tile as tile
from concourse import bacc
from concourse.bass_utils import run_bass_kernel_spmd

B = 16
NCORES = 8
S = B // NCORES          # samples per core
CIN = 320
COUT = 320
H = W = 64
KK = 3
HP = WP = H + 2          # padded spatial
NPIX = H * W             # 4096
NT = 512                 # output free-dim tile: 8 rows x 64 cols
ROWS_PER_NT = NT // W    # 8
NTILES = NPIX // NT      # 8
NG = 2                   # n-tiles per PSUM group
KCH = [(0, 128), (128, 128), (256, 64)]   # CIN chunks (k0, ksize)
MCH = [(0, 128), (128, 128), (256, 64)]   # COUT chunks (m0, msize)

DT_MM = mybir.dt.float32r   # matmul compute dtype (fp32 storage, fast path)
PACK_K = True               # row-pack the CIN=64 remainder chunk tap pairs


def build_nc():
    nc = bacc.Bacc(None, target_bir_lowering=False)
    xpad = nc.dram_tensor("xpad", [S, CIN, HP * WP], DT_MM,
                          kind="ExternalInput")
    wt = nc.dram_tensor("wt", [S, KK * KK, CIN, COUT], DT_MM,
                        kind="ExternalInput")
    bias = nc.dram_tensor("bias", [S, COUT], mybir.dt.float32,
                          kind="ExternalInput")
    y = nc.dram_tensor("y", [S, COUT, NPIX], mybir.dt.float32,
                       kind="ExternalOutput")

    with tile.TileContext(nc) as tc:
        with (
            tc.tile_pool(name="wpool", bufs=1) as wpool,
            tc.tile_pool(name="xpool", bufs=2) as xpool,
            tc.tile_pool(name="bpool", bufs=1) as bpool,
            tc.tile_pool(name="opool", bufs=8) as opool,
            tc.tile_pool(name="psum", bufs=2, space="PSUM") as psum_pool,
        ):
            btile = bpool.tile([128, S * 3], mybir.dt.float32, name="btile",
                               tag="btile")

            # The DMA pool services the SP and ACT HWDGE rings round-robin,
            # so issue loads on ALTERNATING rings in PE-consumption order —
            # round-robin delivery then matches the order the matmuls need
            # the data.
            # Sample 0's startup loads alternate rings in consumption
            # order so round-robin delivery matches the matmul order;
            # sample 1's loads go entirely on the SP ring so the ACT ring
            # stays clear for the epilogue out-DMAs (per-ring FIFO: outs
            # must not queue behind bulk input traffic).
            rings = [nc.sync, nc.scalar]
            rk = [0]

            def dma(out_ap, in_ap, s=0):
                if s == 0:
                    rings[rk[0] % 2].dma_start(out=out_ap, in_=in_ap)
                    rk[0] += 1
                else:
                    nc.sync.dma_start(out=out_ap, in_=in_ap)

            wts_all = {}
            xts_all = {}
            for s in range(S):
                # x in two row-pieces per chunk (Tile tracks subtile deps,
                # so the first n-tile groups unblock on the top piece)
                cut = 34 * WP
                xts = []
                wts = []
                for ci, (c0, cs) in enumerate(KCH):
                    xt = xpool.tile([128, HP * WP], DT_MM,
                                    name=f"x_{s}_{ci}", tag=f"x{ci}")
                    xts.append(xt)
                    wti = wpool.tile([128, KK * KK * COUT], DT_MM,
                                     name=f"wt_{s}_{ci}", tag=f"wt_{s}_{ci}")
                    wts.append(wti)
                for ci, (c0, cs) in enumerate(KCH):
                    dma(xts[ci][:cs, :cut], xpad[s, c0 : c0 + cs, :cut], s)
                    if PACK_K and ci == 2:
                        # upper half: same 64 channels shifted by +1 column,
                        # so row-tile B at window(kh,0) reads tap (kh,1) data
                        dma(xts[ci][64:128, 0 : cut],
                            xpad[s, c0 : c0 + cs, 1 : cut + 1], s)
                    dst = wts[ci][:cs].rearrange("c (t o) -> c t o", o=COUT)
                    src = wt[s].rearrange("t c o -> c t o")[c0 : c0 + cs]
                    # tap 0 first: unblocks the first accumulation slot
                    dma(dst[:, 0:1], src[:, 0:1], s)
                    dma(dst[:, 1:], src[:, 1:], s)
                    if PACK_K and ci == 2:
                        # upper half, at tap-(kh,0) columns: tap (kh,1)
                        # weights for the row-B halves of the k-pairs
                        for kh in range(KK):
                            dma(wts[ci][64 : 64 + cs,
                                        (kh * KK) * COUT :
                                        (kh * KK) * COUT + COUT],
                                wt[s, kh * KK + 1, c0 : c0 + cs], s)
                for ci, (c0, cs) in enumerate(KCH):
                    dma(xts[ci][:cs, cut:], xpad[s, c0 : c0 + cs, cut:], s)
                    if PACK_K and ci == 2:
                        dma(xts[ci][64:128, cut : HP * WP - 1],
                            xpad[s, c0 : c0 + cs, cut + 1 : HP * WP], s)
                xts_all[s] = xts
                wts_all[s] = wts

            for s in range(S):
                for mi, (m0, ms) in enumerate(MCH):
                    nc.scalar.dma_start(
                        out=btile[:ms, s * 3 + mi : s * 3 + mi + 1],
                        in_=bias[s, m0 : m0 + ms],
                    )

            for s in range(S):
                xts = xts_all[s]
                wts = wts_all[s]

                def win(ci, nt_idx, kh, kw, lo, hi):
                    xv = xts[ci].rearrange("p (h w) -> p h w", w=WP)
                    h0 = nt_idx * ROWS_PER_NT
                    return xv[lo:hi, h0 + kh : h0 + kh + ROWS_PER_NT,
                              kw : kw + W]

                for mi, (m0, ms) in enumerate(MCH):
                    for ng in range(NTILES // NG):
                        pa = [psum_pool.tile([128, NT], mybir.dt.float32,
                                             name=f"pa{j}", tag=f"pa{j}")
                              for j in range(NG)]
                        pb = [psum_pool.tile([128, NT], mybir.dt.float32,
                                             name=f"pb{j}", tag=f"pb{j}")
                              for j in range(NG)] if PACK_K else None

                        # full-K chunks
                        first = True
                        for ci in (0, 1):
                            c0, cs = KCH[ci]
                            for t in range(KK * KK):
                                kh, kw = t // KK, t % KK
                                lhsT = wts[ci][:cs, t * COUT + m0 :
                                               t * COUT + m0 + ms]
                                for j in range(NG):
                                    rhs = win(ci, ng * NG + j, kh, kw, 0, cs)
                                    nc.tensor.matmul(pa[j][:ms], lhsT, rhs,
                                                     start=first, stop=False)
                                first = False
                        # remainder chunk (64 rows)
                        c0, cs = KCH[2]
                        for kh in range(KK):
                            for kw in range(KK):
                                t = kh * KK + kw
                                col = t * COUT + m0
                                if PACK_K and kw == 1:
                                    continue  # folded into kw==0's row-B
                                lhsT = wts[2][:cs, col : col + ms]
                                for j in range(NG):
                                    rhs = win(2, ng * NG + j, kh, kw, 0, cs)
                                    last = (kh == KK - 1 and
                                            kw == KK - 1)
                                    nc.tensor.matmul(
                                        pa[j][:ms], lhsT, rhs,
                                        start=False, stop=last,
                                        tile_position=(0, 0))
                                    if PACK_K and kw == 0:
                                        lhsTb = wts[2][64 : 64 + cs,
                                                       col : col + ms]
                                        rhsb = win(2, ng * NG + j, kh, 0,
                                                   64, 64 + cs)
                                        nc.tensor.matmul(
                                            pb[j][:ms], lhsTb, rhsb,
                                            start=(kh == 0),
                                            stop=(kh == KK - 1),
                                            tile_position=(64, 0))

                        for j in range(NG):
                            nt_idx = ng * NG + j
                            ot = opool.tile([128, NT], mybir.dt.float32,
                                            name="ot", tag="ot")
                            bb = btile[:ms, s * 3 + mi : s * 3 + mi + 1]
                            nc.vector.tensor_scalar_add(ot[:ms], pa[j][:ms],
                                                        bb)
                            if PACK_K:
                                nc.vector.tensor_tensor(
                                    out=ot[:ms], in0=ot[:ms],
                                    in1=pb[j][:ms],
                                    op=mybir.AluOpType.add)
                            nc.scalar.dma_start(
                                out=y[s, m0 : m0 + ms,
                                      nt_idx * NT : (nt_idx + 1) * NT],
                                in_=ot[:ms],
                            )
    nc.finalize()
    return nc


def round_fp32r(a):
    """Round fp32 to the PE's fp32r format (11 mantissa bits, RNE).

    Idempotent under the hardware's own input rounding, so pre-rounding on
    the host changes nothing numerically vs letting the PE round."""
    if DT_MM != mybir.dt.float32r:
        return a
    b = a.view(np.uint32)
    r = (b + np.uint32(0x7FF) + ((b >> np.uint32(12)) & np.uint32(1))) \
        & np.uint32(0xFFFFF000)
    return r.view(np.float32)


def prep_inputs(features, weights, bias, class_id):
    f = np.asarray(features, dtype=np.float32)
    w = np.asarray(weights, dtype=np.float32)
    b = np.asarray(bias, dtype=np.float32)
    cid = np.asarray(class_id).astype(np.int64)

    xpad = np.zeros((B, CIN, HP, WP), np.float32)
    xpad[:, :, 1 : H + 1, 1 : W + 1] = round_fp32r(f)
    wsel = w[cid]                                   # [B, COUT, CIN, 3, 3]
    # lhsT layout: [tap, CIN, COUT]
    wt = round_fp32r(np.ascontiguousarray(
        wsel.transpose(0, 3, 4, 2, 1).reshape(B, KK * KK, CIN, COUT)))
    bsel = np.ascontiguousarray(b[cid])             # [B, COUT]

    in_maps = []
    for core in range(NCORES):
        sl = slice(core * S, (core + 1) * S)
        in_maps.append({
            "xpad": np.ascontiguousarray(xpad[sl].reshape(S, CIN, HP * WP)),
            "wt": wt[sl],
            "bias": bsel[sl],
        })
    return in_maps


def run(features, weights, bias, class_id, trace=False):
    in_maps = prep_inputs(features, weights, bias, class_id)
    nc = build_nc()
    last_exc = None
    for attempt in range(4):
        try:
            res = run_bass_kernel_spmd(nc, in_maps,
                                       core_ids=list(range(NCORES)),
                                       trace=trace)
            break
        except Exception as exc:  # transient device faults: retry
            last_exc = exc
            time.sleep(15 * (attempt + 1))
    else:
        raise last_exc
    out = np.concatenate(
        [r["y"].reshape(S, COUT, H, W) for r in res.results], axis=0)
    return out, res


def kernel(features, weights, bias, class_id):
    out, _ = run(features, weights, bias, class_id)
    return out



# revision 3
# speedup vs baseline: 1.4159x; 1.4159x over previous
"""MoE per-sample expert conv3x3 (320->320, 64x64, B=16, 5 experts) on 8 trn2 cores.

Strategy: data-parallel over batch (2 samples/core). Host gathers each
sample's expert weights (weights[class_id]) and prepares THREE column-
shifted copies of the zero-padded input (one per kw tap), each stored
row-contiguous at width 64. The conv becomes shifted matmuls accumulating
in PSUM, with every x window a flat [cs, 128] stride-1 access.

Transposed matmul formulation: output PIXELS are the PE partition dim
(M = 128 = 2 image rows x 64 cols per tile) and COUT=320 is the moving/free
dim (N = 320 <= 512, one PSUM bank, zero column waste). The contraction dim
is CIN in chunks (128, 128, 64); x windows are the stationary operand and
the per-tap weight slabs [CIN, COUT] are the moving operand. This streams
each pixel's contraction exactly once (N=320) instead of the weight-
stationary layout's 3 COUT chunks (N=512 x 3 per 512 pixels with a
half-empty 64-wide chunk).

The CIN=64 remainder chunk packs tap PAIRS into full 128-row matmuls:
  T0 = [ch256:320 @ kw=0 | ch256:320 @ kw=1]  -> (kh,0)+(kh,1) fused, 3x
  T1 = [ch256:320 @ kw=2 | same shifted +1 row] -> (0,2)+(1,2) fused, 1x
leaving only (2,2) at 64 rows: 23 matmuls per pixel-tile instead of 27.
The weight tile's upper partitions hold the partner tap's weights at the
base tap's column slot, so a single matmul accumulates both taps.

Inputs are bf16 (halves DMA + SBUF vs fp32r at the same 1 col/cycle PE
rate); PSUM accumulates fp32. Output staged in SBUF as [pixel, cout] and
DMA'd to a [NPIX, COUT] DRAM tensor (contiguous 1280B rows); the final
transpose to [COUT, H, W] happens on host.

DMA: x/w interleave on the SP + ACT HWDGE rings in PE consumption order;
outputs and bias go on the GPSIMD (Pool) ring so they never queue behind
input traffic. A few zero dummy matmuls at t=0 warm the PE p-state ramp
while the first DMAs land.
"""

import time

import numpy as np
import ml_dtypes

import concourse.bass as bass
import concourse.mybir as mybir
import concourse.tile as tile
from concourse import bacc
from concourse.bass_utils import run_bass_kernel_spmd

B = 16
NCORES = 8
S = B // NCORES          # samples per core
CIN = 320
COUT = 320
H = W = 64
KK = 3
HR = H + 2               # padded rows (top+bottom zero row)
NPIX = H * W             # 4096
FL = HR * W              # flat length of one channel's shifted image (4224)
PTG = 4                  # pixel-tiles per PSUM group (4 tags x 2 bufs = 8 banks)
NGRP = (NPIX // 128) // PTG   # 8 groups/sample; pixel tile = 2 rows x 64 cols
KCH = [(0, 128), (128, 128), (256, 64)]   # CIN chunks (c0, csize)
# x row pieces (row ranges) for progressive dependency unblocking
XP = [(0, 10), (10, 26), (26, 42), (42, 58), (58, 66)]

DT_MM = mybir.dt.bfloat16
NP_MM = ml_dtypes.bfloat16
WARMUP = 8               # dummy matmuls to warm the PE p-state ramp


def build_nc():
    nc = bacc.Bacc(None, target_bir_lowering=False)
    # three kw-shifted, vertically padded copies of the input image
    xsh = nc.dram_tensor("xsh", [S, KK, CIN, FL], DT_MM, kind="ExternalInput")
    wt = nc.dram_tensor("wt", [S, KK * KK, CIN, COUT], DT_MM,
                        kind="ExternalInput")
    biasb = nc.dram_tensor("biasb", [S, 128, COUT], mybir.dt.float32,
                           kind="ExternalInput")
    y = nc.dram_tensor("y", [S, NPIX, COUT], mybir.dt.float32,
                       kind="ExternalOutput")

    with tile.TileContext(nc) as tc:
        with (
            tc.tile_pool(name="wpool", bufs=1) as wpool,
            tc.tile_pool(name="xpool", bufs=2) as xpool,
            tc.tile_pool(name="bpool", bufs=2) as bpool,
            tc.tile_pool(name="opool", bufs=8) as opool,
            tc.tile_pool(name="psum", bufs=2, space="PSUM") as psum_pool,
        ):
            # PE p-state warmup: zero tile via DVE (no DMA dependency), then
            # dummy matmuls that occupy the PE while the first loads land.
            warm = wpool.tile([1, 512], DT_MM, name="warm", tag="warm")
            nc.vector.memset(warm[:, :], 0.0)
            wpsum = psum_pool.tile([128, 512], mybir.dt.float32,
                                   name="wp", tag="p0")
            for _ in range(WARMUP):
                nc.tensor.matmul(wpsum[0:1, :], warm[0:1, 0:1], warm[0:1, :],
                                 start=True, stop=True)

            # bias (pre-broadcast to 128 partitions on host): Pool ring
            bts = []
            for s in range(S):
                bt = bpool.tile([128, COUT], mybir.dt.float32,
                                name=f"bias{s}", tag="bias")
                nc.gpsimd.dma_start(out=bt[:, :], in_=biasb[s])
                bts.append(bt)

            # Input loads alternate the SP / ACT HWDGE rings in PE
            # consumption order so round-robin delivery matches demand.
            rings = [nc.sync, nc.scalar]
            rk = [0]

            def dma(out_ap, in_ap):
                rings[rk[0] % 2].dma_start(out=out_ap, in_=in_ap)
                rk[0] += 1

            xts_all = {}
            wts_all = {}
            for s in range(S):
                # x tiles: [ci0 kw0/1/2, ci1 kw0/1/2, T0, T1]
                xts = []
                for ci in (0, 1):
                    for kw in range(KK):
                        xts.append(xpool.tile([128, FL], DT_MM,
                                              name=f"x_{s}_{ci}_{kw}",
                                              tag=f"x{ci}{kw}"))
                xts.append(xpool.tile([128, FL], DT_MM, name=f"x_{s}_t0",
                                      tag="xt0"))
                xts.append(xpool.tile([128, FL], DT_MM, name=f"x_{s}_t1",
                                      tag="xt1"))
                wts = []
                for ci in range( 3):
                    wts.append(wpool.tile([128, KK * KK * COUT], DT_MM,
                                          name=f"wt_{s}_{ci}",
                                          tag=f"wt_{s}_{ci}"))

                def xpiece(ti, p):
                    a, b = XP[p]
                    if ti < 6:                       # full-chunk kw copies
                        ci, kw = divmod(ti, KK)
                        c0, cs = KCH[ci]
                        dma(xts[ti][:cs, a * W : b * W],
                            xsh[s, kw, c0 : c0 + cs, a * W : b * W])
                    elif ti == 6:                    # T0 = [kw0 | kw1]
                        dma(xts[6][0:64, a * W : b * W],
                            xsh[s, 0, 256:320, a * W : b * W])
                        dma(xts[6][64:128, a * W : b * W],
                            xsh[s, 1, 256:320, a * W : b * W])
                    else:                            # T1 = [kw2 | kw2 +1row]
                        dma(xts[7][0:64, a * W : b * W],
                            xsh[s, 2, 256:320, a * W : b * W])
                        hi = min(b, HR - 1)
                        dma(xts[7][64:128, a * W : hi * W],
                            xsh[s, 2, 256:320, (a + 1) * W : (hi + 1) * W])

                def wslab(ci):
                    c0, cs = KCH[ci]
                    dst = wts[ci][:cs].rearrange("c (t o) -> c t o", o=COUT)
                    src = wt[s].rearrange("t c o -> c t o")[c0 : c0 + cs]
                    dma(dst[:, 0:1], src[:, 0:1])      # tap 0 first
                    if ci == 2:
                        # upper half holds the partner tap's weights at the
                        # base tap's column slot: t0<-t1, t3<-t4, t6<-t7
                        # (fused kh rows) and t2<-t5 (fused (0,2)+(1,2)).
                        dma(wts[2][64:128, 0:COUT], wt[s, 1, c0 : c0 + cs])
                        dma(dst[:, 1:], src[:, 1:])
                        for base, part in ((3, 4), (6, 7), (2, 5)):
                            dma(wts[2][64:128,
                                       base * COUT : (base + 1) * COUT],
                                wt[s, part, c0 : c0 + cs])
                    else:
                        dma(dst[:, 1:], src[:, 1:])

                # startup-critical order: first pieces + weights, then rest
                for ti in (0, 1, 2):
                    xpiece(ti, 0)
                wslab(0)
                for ti in (3, 4, 5):
                    xpiece(ti, 0)
                wslab(1)
                xpiece(6, 0)
                xpiece(7, 0)
                wslab(2)
                for p in range(1, len(XP)):
                    for ti in range(8):
                        xpiece(ti, p)

                xts_all[s] = xts
                wts_all[s] = wts

            for s in range(S):
                xts = xts_all[s]
                wts = wts_all[s]

                # k-instructions: (xtile_idx, w_col_tap, row_off, lo, hi)
                klist = []
                for ci in (0, 1):
                    for t in range(KK * KK):
                        klist.append((ci * KK + t % KK, t, t // KK, 0, 128))
                for kh in range(KK):                 # fused (kh,0)+(kh,1)
                    klist.append((6, kh * KK, kh, 0, 128))
                klist.append((7, 2, 0, 0, 128))      # fused (0,2)+(1,2)
                klist.append((7, 8, 2, 0, 64))       # single (2,2)
                klast = len(klist) - 1

                for g in range(NGRP):
                    ps = [psum_pool.tile([128, 512], mybir.dt.float32,
                                         name=f"ps{j}", tag=f"p{j}")
                          for j in range(PTG)]
                    for ki, (ti, t, ro, lo, hi) in enumerate(klist):
                        wci = 2 if ti >= 6 else ti // KK
                        rhs = wts[wci][lo:hi, t * COUT : (t + 1) * COUT]
                        for j in range(PTG):
                            r0 = (g * PTG + j) * 2
                            o = (r0 + ro) * W
                            lhsT = xts[ti][lo:hi, o : o + 128]
                            nc.tensor.matmul(ps[j][:, :COUT], lhsT, rhs,
                                             start=(ki == 0),
                                             stop=(ki == klast))
                    for j in range(PTG):
                        pt = g * PTG + j
                        ot = opool.tile([128, COUT], mybir.dt.float32,
                                        name="ot", tag="ot")
                        nc.vector.tensor_tensor(
                            out=ot[:, :], in0=ps[j][:, :COUT], in1=bts[s][:, :],
                            op=mybir.AluOpType.add)
                        nc.gpsimd.dma_start(
                            out=y[s, pt * 128 : (pt + 1) * 128, :],
                            in_=ot[:, :])
    nc.finalize()
    return nc


def prep_inputs(features, weights, bias, class_id):
    f = np.asarray(features, dtype=np.float32)
    w = np.asarray(weights, dtype=np.float32)
    b = np.asarray(bias, dtype=np.float32)
    cid = np.asarray(class_id).astype(np.int64)

    pad = np.zeros((B, CIN, HR, W + 2), np.float32)
    pad[:, :, 1 : H + 1, 1 : W + 1] = f
    # three kw-shifted copies, each [B, CIN, HR, W] row-contiguous
    xs = np.stack([pad[:, :, :, k : k + W] for k in range(KK)], axis=1)
    xs = np.ascontiguousarray(xs.reshape(B, KK, CIN, FL)).astype(NP_MM)
    wsel = w[cid]                                   # [B, COUT, CIN, 3, 3]
    # moving-operand layout: [tap, CIN, COUT]
    wtb = np.ascontiguousarray(
        wsel.transpose(0, 3, 4, 2, 1).reshape(B, KK * KK, CIN, COUT)
    ).astype(NP_MM)
    bsel = np.ascontiguousarray(
        np.broadcast_to(b[cid][:, None, :], (B, 128, COUT)))

    in_maps = []
    for core in range(NCORES):
        sl = slice(core * S, (core + 1) * S)
        in_maps.append({
            "xsh": np.ascontiguousarray(xs[sl]),
            "wt": np.ascontiguousarray(wtb[sl]),
            "biasb": np.ascontiguousarray(bsel[sl]),
        })
    return in_maps


def run(features, weights, bias, class_id, trace=False):
    in_maps = prep_inputs(features, weights, bias, class_id)
    nc = build_nc()
    last_exc = None
    for attempt in range(2):
        try:
            res = run_bass_kernel_spmd(nc, in_maps,
                                       core_ids=list(range(NCORES)),
                                       trace=trace)
            break
        except Exception as exc:  # transient device faults: retry
            last_exc = exc
            time.sleep(10)
    else:
        raise last_exc
    out = np.concatenate(
        [r["y"].reshape(S, NPIX, COUT).transpose(0, 2, 1)
          .reshape(S, COUT, H, W)
         for r in res.results], axis=0)
    return np.ascontiguousarray(out), res


def kernel(features, weights, bias, class_id):
    out, _ = run(features, weights, bias, class_id)
    return out


# revision 17
# speedup vs baseline: 1.4471x; 1.0220x over previous
"""MoE per-sample expert conv3x3 (320->320, 64x64, B=16, 5 experts) on 8 trn2 cores.

Strategy: data-parallel over batch (2 samples/core). Host gathers each
sample's expert weights (weights[class_id]) and prepares the exact SBUF
image layouts in DRAM so device DMAs are few and large (descriptor
generation is a single serial device, so DMA count is a real resource).

Transposed matmul formulation: output PIXELS are the PE partition dim
(M = 128 = 2 image rows x 64 cols per tile) and COUT=320 is the moving/free
dim (N = 320 <= 512, one PSUM bank, zero column waste). The contraction dim
is CIN in chunks (128, 128, 64); x windows are the stationary operand and
the per-tap weight slabs [CIN, COUT] are the moving operand. Every x window
is a flat [cs, 128] stride-1 slice of a per-sample SBUF megatile holding 8
sub-images of 66 rows x 64 cols (vertically zero-padded, one copy per kw
column shift):

  ti 0-2: CIN 0:128   at kw = 0,1,2      ti 3-5: CIN 128:256 at kw = 0,1,2
  ti 6:   [CIN 256:320 @ kw0 | @ kw1]    ti 7:   [CIN 256:320 @ kw2 | @ kw2
                                                  shifted one image row]

The CIN=64 remainder chunk packs tap PAIRS into full 128-row matmuls using
ti 6 ((kh,0)+(kh,1) fused, x3) and ti 7 ((0,2)+(1,2) fused, x1), leaving
only (2,2) at 64 rows: 23 matmuls per pixel-tile. The ci2 weight tile's
upper partitions hold the partner tap's weights at the base tap's column
slot (host-prepacked, one DMA).

Inputs are bf16 (halves DMA + SBUF vs fp32r at the same 1 col/cycle PE
rate); PSUM accumulates fp32. Output staged in SBUF as [pixel, cout] and
DMA'd to a [NPIX, COUT] DRAM tensor (contiguous 1280B rows); the final
transpose to [COUT, H, W] happens on host.

Schedule: groups of 4 pixel-tiles accumulate in 4 PSUM banks (x2 buffered =
all 8 banks), contraction-major so the 4 tiles finish together and their
epilogues (DVE bias-add from PSUM, then output DMA on the Pool ring)
overlap the next group. The globally-last group runs pixel-tile-major with
its outputs fanned across rings so the drain tail is one epilogue deep.
A few zero dummy matmuls at t=0 warm the PE p-state ramp while the first
DMAs land.
"""

import time

import numpy as np
import ml_dtypes

import concourse.bass as bass
import concourse.mybir as mybir
import concourse.tile as tile
from concourse import bacc
from concourse.bass_utils import run_bass_kernel_spmd

B = 16
NCORES = 8
S = B // NCORES          # samples per core
CIN = 320
COUT = 320
H = W = 64
KK = 3
HR = H + 2               # padded rows (top+bottom zero row)
NPIX = H * W             # 4096
FL = HR * W              # flat length of one sub-image (4224)
NT = 8                   # sub-images per sample
PTG = 4                  # pixel-tiles per PSUM group (4 tags x 2 bufs = 8 banks)
NGRP = (NPIX // 128) // PTG   # 8 groups/sample; pixel tile = 2 rows x 64 cols
KCH = [(0, 128), (128, 128), (256, 64)]   # CIN chunks (c0, csize)
# x row pieces (row ranges) for progressive dependency unblocking
XP = [(0, 10), (10, 26), (26, 42), (42, 58), (58, 66)]

DT_MM = mybir.dt.bfloat16
NP_MM = ml_dtypes.bfloat16
WARMUP = 7               # dummy matmuls to warm the PE p-state ramp


def build_nc():
    nc = bacc.Bacc(None, target_bir_lowering=False)
    # 8 kw-shifted / composite sub-images per sample, partition-major
    xall = nc.dram_tensor("xall", [S, 128, NT, FL], DT_MM,
                          kind="ExternalInput")
    wt = nc.dram_tensor("wt", [S, KK * KK, CIN, COUT], DT_MM,
                        kind="ExternalInput")
    # ci2 weight tile, host-prepacked [128, 9*COUT]
    wt2p = nc.dram_tensor("wt2p", [S, 128, KK * KK * COUT], DT_MM,
                          kind="ExternalInput")
    biasb = nc.dram_tensor("biasb", [S, 128, COUT], mybir.dt.float32,
                           kind="ExternalInput")
    y = nc.dram_tensor("y", [S, NPIX, COUT], mybir.dt.float32,
                       kind="ExternalOutput")

    with tile.TileContext(nc) as tc:
        with (
            tc.tile_pool(name="wpool", bufs=1) as wpool,
            tc.tile_pool(name="xpool", bufs=2) as xpool,
            tc.tile_pool(name="bpool", bufs=2) as bpool,
            tc.tile_pool(name="opool", bufs=8) as opool,
            tc.tile_pool(name="psum", bufs=2, space="PSUM") as psum_pool,
        ):
            # PE p-state warmup: zero tile via DVE (no DMA dependency), then
            # dummy matmuls that occupy the PE while the first loads land.
            warm = wpool.tile([1, 512], DT_MM, name="warm", tag="warm")
            nc.vector.memset(warm[:, :], 0.0)
            wpsum = psum_pool.tile([128, 512], mybir.dt.float32,
                                   name="wp", tag="p0")
            for _ in range(WARMUP):
                nc.tensor.matmul(wpsum[0:1, :], warm[0:1, 0:1], warm[0:1, :],
                                 start=True, stop=True)

            # Input loads alternate the SP / ACT HWDGE rings in PE
            # consumption order so round-robin delivery matches demand.
            rings = [nc.sync, nc.scalar]
            rk = [0]

            def dma(out_ap, in_ap):
                rings[rk[0] % 2].dma_start(out=out_ap, in_=in_ap)
                rk[0] += 1

            xbs_all = {}
            wts_all = {}
            for s in range(S):
                xb = xpool.tile([128, NT * FL], DT_MM, name=f"x_{s}",
                                tag="xb")
                xbv = xb.rearrange("p (t f) -> p t f", f=FL)
                wts = [wpool.tile([128, KK * KK * COUT], DT_MM,
                                  name=f"wt_{s}_{ci}", tag=f"wt_{s}_{ci}")
                       for ci in range(3)]

                def xpiece(t0, t1, p):
                    a, b = XP[p]
                    dma(xbv[:, t0:t1, a * W : b * W],
                        xall[s][:, t0:t1, a * W : b * W])

                def wslab(ci, part):
                    # part 0: tap 0; part 1: taps 1-4; part 2: taps 5-8
                    c0, cs = KCH[ci]
                    dst = wts[ci][:cs].rearrange("c (t o) -> c t o", o=COUT)
                    src = wt[s].rearrange("t c o -> c t o")[c0 : c0 + cs]
                    t0, t1 = ((0, 1), (1, 5), (5, 9))[part]
                    dma(dst[:, t0:t1], src[:, t0:t1])

                # startup-critical order: interleave first pieces + weights
                # in PE consumption order (HWDGE generation is one shared
                # serial device, so stream order is delivery order).
                xpiece(0, 3, 0)
                wslab(0, 0)
                wslab(0, 1)
                wslab(0, 2)
                xpiece(3, 6, 0)
                wslab(1, 0)
                wslab(1, 1)
                wslab(1, 2)
                xpiece(6, 8, 0)
                dma(wts[2][:, :], wt2p[s])
                for p in range(1, len(XP)):
                    xpiece(0, 3, p)
                    xpiece(3, 6, p)
                    xpiece(6, 8, p)

                xbs_all[s] = xb
                wts_all[s] = wts

            # bias (pre-broadcast to 128 partitions on host): Pool ring.
            # Issued after the startup-critical input loads — first use is
            # the first epilogue at ~14us.
            bts = []
            for s in range(S):
                bt = bpool.tile([128, COUT], mybir.dt.float32,
                                name=f"bias{s}", tag="bias")
                nc.gpsimd.dma_start(out=bt[:, :], in_=biasb[s])
                bts.append(bt)

            for s in range(S):
                xb = xbs_all[s]
                wts = wts_all[s]

                # k-instructions: (xtile_idx, w_col_tap, row_off, lo, hi)
                klist = []
                for ci in (0, 1):
                    for t in range(KK * KK):
                        klist.append((ci * KK + t % KK, t, t // KK, 0, 128))
                for kh in range(KK):                 # fused (kh,0)+(kh,1)
                    klist.append((6, kh * KK, kh, 0, 128))
                klist.append((7, 2, 0, 0, 128))      # fused (0,2)+(1,2)
                klist.append((7, 8, 2, 0, 64))       # single (2,2)
                klast = len(klist) - 1

                def mm(ps_j, pt, ki):
                    ti, t, ro, lo, hi = klist[ki]
                    wci = 2 if ti >= 6 else ti // KK
                    rhs = wts[wci][lo:hi, t * COUT : (t + 1) * COUT]
                    o = ti * FL + (pt * 2 + ro) * W
                    lhsT = xb[lo:hi, o : o + 128]
                    nc.tensor.matmul(ps_j[:, :COUT], lhsT, rhs,
                                     start=(ki == 0), stop=(ki == klast))

                def epi(ps_j, pt, ring):
                    ot = opool.tile([128, COUT], mybir.dt.float32,
                                    name="ot", tag="ot")
                    nc.vector.tensor_tensor(
                        out=ot[:, :], in0=ps_j[:, :COUT], in1=bts[s][:, :],
                        op=mybir.AluOpType.add)
                    ring.dma_start(out=y[s, pt * 128 : (pt + 1) * 128, :],
                                   in_=ot[:, :])

                for g in range(NGRP):
                    ps = [psum_pool.tile([128, 512], mybir.dt.float32,
                                         name=f"ps{j}", tag=f"p{j}")
                          for j in range(PTG)]
                    if s == S - 1 and g == NGRP - 1:
                        # final group: pixel-tile-major so each epilogue
                        # overlaps the next tile's matmuls, and each output
                        # DMA gets its own ring — the drain tail is one
                        # epilogue instead of four serialized ones.
                        rings_o = [nc.gpsimd, nc.scalar, nc.gpsimd, nc.sync]
                        for j in range(PTG):
                            for ki in range(len(klist)):
                                mm(ps[j], g * PTG + j, ki)
                            epi(ps[j], g * PTG + j, rings_o[j % 4])
                    else:
                        for ki in range(len(klist)):
                            for j in range(PTG):
                                mm(ps[j], g * PTG + j, ki)
                        for j in range(PTG):
                            epi(ps[j], g * PTG + j, nc.gpsimd)
    nc.finalize()
    return nc


def prep_inputs(features, weights, bias, class_id):
    f = np.asarray(features, dtype=np.float32)
    w = np.asarray(weights, dtype=np.float32)
    b = np.asarray(bias, dtype=np.float32)
    cid = np.asarray(class_id).astype(np.int64)

    pad = np.zeros((B, CIN, HR, W + 2), np.float32)
    pad[:, :, 1 : H + 1, 1 : W + 1] = f
    # kw-shifted copies, each [B, CIN, HR, W] row-contiguous
    xs = np.stack([pad[:, :, :, k : k + W] for k in range(KK)],
                  axis=1).reshape(B, KK, CIN, FL)
    # 8 sub-images, partition-major [B, 128, 8, FL]
    xa = np.zeros((B, 128, NT, FL), np.float32)
    for ci in range(2):
        for kw in range(KK):
            xa[:, :, ci * KK + kw] = xs[:, kw, ci * 128 : (ci + 1) * 128]
    xa[:, 0:64, 6] = xs[:, 0, 256:320]
    xa[:, 64:128, 6] = xs[:, 1, 256:320]
    xa[:, 0:64, 7] = xs[:, 2, 256:320]
    # upper ti7: kw2 shifted one image row (row r holds image row r+1)
    xa[:, 64:128, 7, : FL - W] = xs[:, 2, 256:320, W:]
    xa = xa.astype(NP_MM)

    wsel = w[cid]                                   # [B, COUT, CIN, 3, 3]
    # moving-operand layout: [tap, CIN, COUT]
    wtb = np.ascontiguousarray(
        wsel.transpose(0, 3, 4, 2, 1).reshape(B, KK * KK, CIN, COUT))
    # ci2 weight tile [128, 9*COUT]: lower = taps at their slots, upper =
    # partner tap at the base slot (t0<-t1, t3<-t4, t6<-t7, t2<-t5)
    w2 = np.zeros((B, 128, KK * KK, COUT), np.float32)
    w2[:, 0:64] = wtb[:, :, 256:320].transpose(0, 2, 1, 3)
    for base, part in ((0, 1), (3, 4), (6, 7), (2, 5)):
        w2[:, 64:128, base] = wtb[:, part, 256:320]
    w2 = w2.reshape(B, 128, KK * KK * COUT).astype(NP_MM)
    wtb = wtb.astype(NP_MM)

    bsel = np.ascontiguousarray(
        np.broadcast_to(b[cid][:, None, :], (B, 128, COUT)))

    in_maps = []
    for core in range(NCORES):
        sl = slice(core * S, (core + 1) * S)
        in_maps.append({
            "xall": np.ascontiguousarray(xa[sl]),
            "wt": np.ascontiguousarray(wtb[sl]),
            "wt2p": np.ascontiguousarray(w2[sl]),
            "biasb": np.ascontiguousarray(bsel[sl]),
        })
    return in_maps


def run(features, weights, bias, class_id, trace=False):
    in_maps = prep_inputs(features, weights, bias, class_id)
    nc = build_nc()
    last_exc = None
    for attempt in range(2):
        try:
            res = run_bass_kernel_spmd(nc, in_maps,
                                       core_ids=list(range(NCORES)),
                                       trace=trace)
            break
        except Exception as exc:  # transient device faults: retry
            last_exc = exc
            time.sleep(10)
    else:
        raise last_exc
    out = np.concatenate(
        [r["y"].reshape(S, NPIX, COUT).transpose(0, 2, 1)
          .reshape(S, COUT, H, W)
         for r in res.results], axis=0)
    return np.ascontiguousarray(out), res


def kernel(features, weights, bias, class_id):
    out, _ = run(features, weights, bias, class_id)
    return out


# revision 21
# speedup vs baseline: 1.7451x; 1.2060x over previous
"""MoE per-sample expert conv3x3 on 8 trn2 cores — fp8 DoubleRow variant.

Same transposed formulation as kernel.py (pixels = PE partitions, COUT=320
= moving free dim), but the contraction runs in fp8-e4m3 DoubleRow mode:
each matmul contracts TWO 128-row planes (plane stride = one sub-image) at
0.5 cycles per output column — 4x the bf16 row throughput.

Precision: residual split x = x_hi + x_lo, w = w_hi + w_lo (all e4m3, w
pre-scaled by 64 so w_hi/w_lo stay in e4m3's normal range; the 64 comes
back out on host). Three passes accumulate x_hi*w_hi + x_lo*w_hi +
x_hi*w_lo in fp32 PSUM; the dropped x_lo*w_lo term is O(eps^2) ~ 2e-3
relative. Measured end-to-end error ~4e-3 vs the 2e-2 gate.

Layout: 16 sub-images (8 hi + 8 lo) of 66 rows x 64 cols per sample in one
SBUF megatile:
  s0-s2: CIN 0:128   @ kw=0,1,2     s3-s5: CIN 128:256 @ kw=0,1,2
  s6:    [CIN 256:320 @ kw0 | @ kw1]
  s7:    [CIN 256:320 @ kw2 | @ kw2 shifted one image row]
DoubleRow pairs are adjacent sub-images (0,1) (2,3) (4,5) (6,7) at plane
stride FL; the window row offset ro=kh is shared by both planes. Per pass
and ro that is 4 instructions; 3 passes x 3 ro x 4 = 36 per pixel-tile
(5760 PE cycles vs 7360 for the bf16 kernel). Tap/channel assignment,
including ZERO weights for duplicate or invalid plane slots (C2 covers
(0,2)+(1,2) at ro=0 and (2,2) at ro=1-upper; everything else zero), is
host-packed into a per-instruction [128, 2, 320] fp8 weight block — the
device issues 36 uniform matmuls.

Everything else (schedule, rings, epilogue, warmup) matches kernel.py.
"""

import time

import numpy as np
import ml_dtypes

import concourse.bass as bass
import concourse.mybir as mybir
import concourse.tile as tile
from concourse import bacc
from concourse.bass_utils import run_bass_kernel_spmd

B = 16
NCORES = 8
S = B // NCORES
CIN = 320
COUT = 320
H = W = 64
KK = 3
HR = H + 2
NPIX = H * W
FL = HR * W              # 4224
NSUB = 8                 # sub-images per precision half
NPASS = 3                # hh, lh, hl
NPAIR = 4
NI = NPASS * KK * NPAIR  # 36 DoubleRow instructions per pixel-tile
PTG = 4
NGRP = (NPIX // 128) // PTG
XP = [(0, 10), (10, 26), (26, 42), (42, 58), (58, 66)]

DT8 = mybir.dt.float8e4
NP8 = ml_dtypes.float8_e4m3
WSCALE = 64.0
WARMUP = 7


def build_nc():
    nc = bacc.Bacc(None, target_bir_lowering=False)
    # [S, 128, hi/lo, 8 sub-images, FL]
    xall = nc.dram_tensor("xall", [S, 128, 2, NSUB, FL], DT8,
                          kind="ExternalInput")
    # [S, 128, 36 instr blocks x 2 planes x COUT]
    w8d = nc.dram_tensor("w8d", [S, 128, NI * 2 * COUT], DT8,
                         kind="ExternalInput")
    biasb = nc.dram_tensor("biasb", [S, 128, COUT], mybir.dt.float32,
                           kind="ExternalInput")
    y = nc.dram_tensor("y", [S, NPIX, COUT], mybir.dt.float32,
                       kind="ExternalOutput")

    with tile.TileContext(nc) as tc:
        with (
            tc.tile_pool(name="wpool", bufs=1) as wpool,
            tc.tile_pool(name="xpool", bufs=2) as xpool,
            tc.tile_pool(name="bpool", bufs=2) as bpool,
            tc.tile_pool(name="opool", bufs=8) as opool,
            tc.tile_pool(name="psum", bufs=2, space="PSUM") as psum_pool,
        ):
            warm = wpool.tile([1, 512], mybir.dt.bfloat16, name="warm",
                              tag="warm")
            nc.vector.memset(warm[:, :], 0.0)
            wpsum = psum_pool.tile([128, 512], mybir.dt.float32,
                                   name="wp", tag="p0")
            for _ in range(WARMUP):
                nc.tensor.matmul(wpsum[0:1, :], warm[0:1, 0:1], warm[0:1, :],
                                 start=True, stop=True)

            rings = [nc.sync, nc.scalar]
            rk = [0]

            def dma(out_ap, in_ap):
                rings[rk[0] % 2].dma_start(out=out_ap, in_=in_ap)
                rk[0] += 1

            xbs_all = {}
            wts_all = {}
            for s in range(S):
                xb = xpool.tile([128, 2 * NSUB * FL], DT8, name=f"x_{s}",
                                tag="xb")
                xbv = xb.rearrange("p (h t f) -> p h t f", h=2, f=FL)
                w8 = wpool.tile([128, NI * 2 * COUT], DT8, name=f"w_{s}",
                                tag=f"w_{s}")

                def xpiece(t0, t1, p, halves=(0, 1)):
                    a, b = XP[p]
                    for h in halves:
                        dma(xbv[:, h, t0:t1, a * W : b * W],
                            xall[s][:, h, t0:t1, a * W : b * W])

                def wpass(p):
                    # pass p's 12 instruction blocks
                    c = 12 * 2 * COUT
                    dma(w8[:, p * c : (p + 1) * c],
                        w8d[s][:, p * c : (p + 1) * c])

                xpiece(0, 3, 0)
                wpass(0)
                xpiece(3, 6, 0)
                wpass(1)
                xpiece(6, 8, 0)
                wpass(2)
                for p in range(1, len(XP)):
                    xpiece(0, 3, p)
                    xpiece(3, 6, p)
                    xpiece(6, 8, p)

                xbs_all[s] = xb
                wts_all[s] = w8

            bts = []
            for s in range(S):
                bt = bpool.tile([128, COUT], mybir.dt.float32,
                                name=f"bias{s}", tag="bias")
                nc.gpsimd.dma_start(out=bt[:, :], in_=biasb[s])
                bts.append(bt)

            for s in range(S):
                xbv = xbs_all[s].rearrange("p (t f) -> p t f", f=FL)
                w8v = wts_all[s].rearrange("p (i two o) -> p i two o",
                                           two=2, o=COUT)

                # (x half: 0=hi, 1=lo) per pass: hh, lh, hl
                XHALF = (0, 1, 0)
                klist = [(p, ro, m) for p in range(NPASS)
                         for ro in range(KK) for m in range(NPAIR)]
                klast = len(klist) - 1

                def mm(ps_j, pt, ki):
                    p, ro, m = klist[ki]
                    tb = XHALF[p] * NSUB + 2 * m
                    o = (pt * 2 + ro) * W
                    lhsT = xbv[:, tb : tb + 2, o : o + 128]
                    rhs = w8v[:, p * 12 + ro * NPAIR + m]
                    nc.tensor.matmul(ps_j[:, :COUT], lhsT, rhs,
                                     start=(ki == 0), stop=(ki == klast),
                                     perf_mode=mybir.MatmulPerfMode.DoubleRow)

                def epi(ps_j, pt, ring):
                    ot = opool.tile([128, COUT], mybir.dt.float32,
                                    name="ot", tag="ot")
                    nc.vector.tensor_tensor(
                        out=ot[:, :], in0=ps_j[:, :COUT], in1=bts[s][:, :],
                        op=mybir.AluOpType.add)
                    ring.dma_start(out=y[s, pt * 128 : (pt + 1) * 128, :],
                                   in_=ot[:, :])

                for g in range(NGRP):
                    ps = [psum_pool.tile([128, 512], mybir.dt.float32,
                                         name=f"ps{j}", tag=f"p{j}")
                          for j in range(PTG)]
                    if s == S - 1 and g == NGRP - 1:
                        rings_o = [nc.gpsimd, nc.scalar, nc.gpsimd, nc.sync]
                        for j in range(PTG):
                            for ki in range(len(klist)):
                                mm(ps[j], g * PTG + j, ki)
                            epi(ps[j], g * PTG + j, rings_o[j % 4])
                    else:
                        for ki in range(len(klist)):
                            for j in range(PTG):
                                mm(ps[j], g * PTG + j, ki)
                        for j in range(PTG):
                            epi(ps[j], g * PTG + j, nc.gpsimd)
    nc.finalize()
    return nc


def prep_inputs(features, weights, bias, class_id):
    f = np.asarray(features, dtype=np.float32)
    w = np.asarray(weights, dtype=np.float32)
    b = np.asarray(bias, dtype=np.float32)
    cid = np.asarray(class_id).astype(np.int64)

    pad = np.zeros((B, CIN, HR, W + 2), np.float32)
    pad[:, :, 1 : H + 1, 1 : W + 1] = f
    xs = np.stack([pad[:, :, :, k : k + W] for k in range(KK)],
                  axis=1).reshape(B, KK, CIN, FL)
    xhi = xs.astype(NP8)
    xlo = (xs - xhi.astype(np.float32)).astype(NP8)

    xa = np.zeros((B, 128, 2, NSUB, FL), NP8)
    for h, src in enumerate((xhi, xlo)):
        for ci in range(2):
            for kw in range(KK):
                xa[:, :, h, ci * KK + kw] = src[:, kw,
                                                ci * 128 : (ci + 1) * 128]
        xa[:, 0:64, h, 6] = src[:, 0, 256:320]
        xa[:, 64:128, h, 6] = src[:, 1, 256:320]
        xa[:, 0:64, h, 7] = src[:, 2, 256:320]
        xa[:, 64:128, h, 7, : FL - W] = src[:, 2, 256:320, W:]

    wsel = w[cid]                                   # [B, COUT, CIN, 3, 3]
    wtb = np.ascontiguousarray(
        wsel.transpose(0, 3, 4, 2, 1).reshape(B, KK * KK, CIN, COUT))
    wsc = wtb * WSCALE
    whi_8 = wsc.astype(NP8)
    wlo_8 = (wsc - whi_8.astype(np.float32)).astype(NP8)

    def t(kh, kw):
        return kh * KK + kw

    # per-instruction weight blocks [B, 128, NI, 2, COUT]
    w8 = np.zeros((B, 128, NI, 2, COUT), NP8)
    for p, wsrc in enumerate((whi_8, whi_8, wlo_8)):
        for ro in range(KK):
            for m in range(NPAIR):
                ii = p * 12 + ro * NPAIR + m
                if m == 0:
                    w8[:, :, ii, 0] = wsrc[:, t(ro, 0), 0:128]
                    w8[:, :, ii, 1] = wsrc[:, t(ro, 1), 0:128]
                elif m == 1:
                    w8[:, :, ii, 0] = wsrc[:, t(ro, 2), 0:128]
                    w8[:, :, ii, 1] = wsrc[:, t(ro, 0), 128:256]
                elif m == 2:
                    w8[:, :, ii, 0] = wsrc[:, t(ro, 1), 128:256]
                    w8[:, :, ii, 1] = wsrc[:, t(ro, 2), 128:256]
                else:
                    w8[:, 0:64, ii, 0] = wsrc[:, t(ro, 0), 256:320]
                    w8[:, 64:128, ii, 0] = wsrc[:, t(ro, 1), 256:320]
                    if ro == 0:
                        w8[:, 0:64, ii, 1] = wsrc[:, t(0, 2), 256:320]
                        w8[:, 64:128, ii, 1] = wsrc[:, t(1, 2), 256:320]
                    elif ro == 1:
                        w8[:, 64:128, ii, 1] = wsrc[:, t(2, 2), 256:320]
    w8 = w8.reshape(B, 128, NI * 2 * COUT)

    bsel = np.ascontiguousarray(
        np.broadcast_to(WSCALE * b[cid][:, None, :], (B, 128, COUT)))

    in_maps = []
    for core in range(NCORES):
        sl = slice(core * S, (core + 1) * S)
        in_maps.append({
            "xall": np.ascontiguousarray(xa[sl]),
            "w8d": np.ascontiguousarray(w8[sl]),
            "biasb": np.ascontiguousarray(bsel[sl]),
        })
    return in_maps


def run(features, weights, bias, class_id, trace=False):
    in_maps = prep_inputs(features, weights, bias, class_id)
    nc = build_nc()
    last_exc = None
    for attempt in range(2):
        try:
            res = run_bass_kernel_spmd(nc, in_maps,
                                       core_ids=list(range(NCORES)),
                                       trace=trace)
            break
        except Exception as exc:
            last_exc = exc
            time.sleep(10)
    else:
        raise last_exc
    out = np.concatenate(
        [r["y"].reshape(S, NPIX, COUT).transpose(0, 2, 1)
          .reshape(S, COUT, H, W)
         for r in res.results], axis=0)
    return np.ascontiguousarray(out / WSCALE), res


def kernel(features, weights, bias, class_id):
    out, _ = run(features, weights, bias, class_id)
    return out


# revision 22
# speedup vs baseline: 1.7948x; 1.0285x over previous
"""MoE per-sample expert conv3x3 on 8 trn2 cores — fp8 DoubleRow variant.

Same transposed formulation as kernel.py (pixels = PE partitions, COUT=320
= moving free dim), but the contraction runs in fp8-e4m3 DoubleRow mode:
each matmul contracts TWO 128-row planes (plane stride = one sub-image) at
0.5 cycles per output column — 4x the bf16 row throughput.

Precision: residual split x = x_hi + x_lo, w = w_hi + w_lo (all e4m3, w
pre-scaled by 64 so w_hi/w_lo stay in e4m3's normal range; the 64 comes
back out on host). Three passes accumulate x_hi*w_hi + x_lo*w_hi +
x_hi*w_lo in fp32 PSUM; the dropped x_lo*w_lo term is O(eps^2) ~ 2e-3
relative. Measured end-to-end error ~4e-3 vs the 2e-2 gate.

Layout: 16 sub-images (8 hi + 8 lo) of 66 rows x 64 cols per sample in one
SBUF megatile:
  s0-s2: CIN 0:128   @ kw=0,1,2     s3-s5: CIN 128:256 @ kw=0,1,2
  s6:    [CIN 256:320 @ kw0 | @ kw1]
  s7:    [CIN 256:320 @ kw2 | @ kw2 shifted one image row]
DoubleRow pairs are adjacent sub-images (0,1) (2,3) (4,5) (6,7) at plane
stride FL; the window row offset ro=kh is shared by both planes. Per pass
and ro that is 4 instructions; 3 passes x 3 ro x 4 = 36 per pixel-tile
(5760 PE cycles vs 7360 for the bf16 kernel). Tap/channel assignment,
including ZERO weights for duplicate or invalid plane slots (C2 covers
(0,2)+(1,2) at ro=0 and (2,2) at ro=1-upper; everything else zero), is
host-packed into a per-instruction [128, 2, 320] fp8 weight block — the
device issues 36 uniform matmuls.

Everything else (schedule, rings, epilogue, warmup) matches kernel.py.
"""

import time

import numpy as np
import ml_dtypes

import concourse.bass as bass
import concourse.mybir as mybir
import concourse.tile as tile
from concourse import bacc
from concourse.bass_utils import run_bass_kernel_spmd

B = 16
NCORES = 8
S = B // NCORES
CIN = 320
COUT = 320
H = W = 64
KK = 3
HR = H + 2
NPIX = H * W
FL = HR * W              # 4224
NSUB = 8                 # sub-images per precision half
NPASS = 3                # hh, lh, hl (hh and lh share weight blocks)
NPAIR = 4
NI = NPASS * KK * NPAIR  # 36 DoubleRow instructions per pixel-tile
NWB = 2 * KK * NPAIR     # 24 stored weight blocks (w_hi + w_lo sets)
PTG = 4
NGRP = (NPIX // 128) // PTG
XP = [(0, 10), (10, 26), (26, 42), (42, 58), (58, 66)]

DT8 = mybir.dt.float8e4
NP8 = ml_dtypes.float8_e4m3
WSCALE = 64.0
WARMUP = 7


def build_nc():
    nc = bacc.Bacc(None, target_bir_lowering=False)
    # [S, 128, hi/lo, 8 sub-images, FL]
    xall = nc.dram_tensor("xall", [S, 128, 2, NSUB, FL], DT8,
                          kind="ExternalInput")
    # [S, 128, 24 weight blocks x 2 planes x COUT]
    w8d = nc.dram_tensor("w8d", [S, 128, NWB * 2 * COUT], DT8,
                         kind="ExternalInput")
    biasb = nc.dram_tensor("biasb", [S, 128, COUT], mybir.dt.float32,
                           kind="ExternalInput")
    y = nc.dram_tensor("y", [S, NPIX, COUT], mybir.dt.float32,
                       kind="ExternalOutput")

    with tile.TileContext(nc) as tc:
        with (
            tc.tile_pool(name="wpool", bufs=1) as wpool,
            tc.tile_pool(name="xpool", bufs=2) as xpool,
            tc.tile_pool(name="bpool", bufs=2) as bpool,
            tc.tile_pool(name="opool", bufs=8) as opool,
            tc.tile_pool(name="psum", bufs=2, space="PSUM") as psum_pool,
        ):
            warm = wpool.tile([1, 512], mybir.dt.bfloat16, name="warm",
                              tag="warm")
            nc.vector.memset(warm[:, :], 0.0)
            wpsum = psum_pool.tile([128, 512], mybir.dt.float32,
                                   name="wp", tag="p0")
            for _ in range(WARMUP):
                nc.tensor.matmul(wpsum[0:1, :], warm[0:1, 0:1], warm[0:1, :],
                                 start=True, stop=True)

            rings = [nc.sync, nc.scalar]
            rk = [0]

            def dma(out_ap, in_ap):
                rings[rk[0] % 2].dma_start(out=out_ap, in_=in_ap)
                rk[0] += 1

            xbs_all = {}
            wts_all = {}
            for s in range(S):
                xb = xpool.tile([128, 2 * NSUB * FL], DT8, name=f"x_{s}",
                                tag="xb")
                xbv = xb.rearrange("p (h t f) -> p h t f", h=2, f=FL)
                w8 = wpool.tile([128, NWB * 2 * COUT], DT8, name=f"w_{s}",
                                tag=f"w_{s}")

                def xpiece(t0, t1, p, halves=(0, 1)):
                    a, b = XP[p]
                    for h in halves:
                        dma(xbv[:, h, t0:t1, a * W : b * W],
                            xall[s][:, h, t0:t1, a * W : b * W])

                def wchunk(half, ro):
                    # one ro-row of 4 blocks from the hi (0) or lo (1) set
                    c = NPAIR * 2 * COUT
                    o = (half * KK + ro) * c
                    dma(w8[:, o : o + c], w8d[s][:, o : o + c])

                # demand order: hh pass (hi x + hi w), lh (lo x, same w),
                # hl (hi x again, lo w)
                xpiece(0, 3, 0, (0,))
                wchunk(0, 0)
                xpiece(3, 6, 0, (0,))
                wchunk(0, 1)
                xpiece(6, 8, 0, (0,))
                wchunk(0, 2)
                xpiece(0, 3, 0, (1,))
                xpiece(3, 6, 0, (1,))
                xpiece(6, 8, 0, (1,))
                wchunk(1, 0)
                wchunk(1, 1)
                wchunk(1, 2)
                for p in range(1, len(XP)):
                    xpiece(0, 3, p)
                    xpiece(3, 6, p)
                    xpiece(6, 8, p)

                xbs_all[s] = xb
                wts_all[s] = w8

            bts = []
            for s in range(S):
                bt = bpool.tile([128, COUT], mybir.dt.float32,
                                name=f"bias{s}", tag="bias")
                nc.gpsimd.dma_start(out=bt[:, :], in_=biasb[s])
                bts.append(bt)

            for s in range(S):
                xbv = xbs_all[s].rearrange("p (t f) -> p t f", f=FL)
                w8v = wts_all[s].rearrange("p (i two o) -> p i two o",
                                           two=2, o=COUT)

                # (x half: 0=hi, 1=lo) per pass: hh, lh, hl
                XHALF = (0, 1, 0)
                klist = [(p, ro, m) for p in range(NPASS)
                         for ro in range(KK) for m in range(NPAIR)]
                klast = len(klist) - 1

                WHALF = (0, 0, 1)   # hh and lh share the w_hi blocks

                def mm(ps_j, pt, ki):
                    p, ro, m = klist[ki]
                    tb = XHALF[p] * NSUB + 2 * m
                    o = (pt * 2 + ro) * W
                    lhsT = xbv[:, tb : tb + 2, o : o + 128]
                    rhs = w8v[:, (WHALF[p] * KK + ro) * NPAIR + m]
                    nc.tensor.matmul(ps_j[:, :COUT], lhsT, rhs,
                                     start=(ki == 0), stop=(ki == klast),
                                     perf_mode=mybir.MatmulPerfMode.DoubleRow)

                def epi(ps_j, pt, ring):
                    ot = opool.tile([128, COUT], mybir.dt.float32,
                                    name="ot", tag="ot")
                    nc.vector.tensor_tensor(
                        out=ot[:, :], in0=ps_j[:, :COUT], in1=bts[s][:, :],
                        op=mybir.AluOpType.add)
                    ring.dma_start(out=y[s, pt * 128 : (pt + 1) * 128, :],
                                   in_=ot[:, :])

                for g in range(NGRP):
                    ps = [psum_pool.tile([128, 512], mybir.dt.float32,
                                         name=f"ps{j}", tag=f"p{j}")
                          for j in range(PTG)]
                    if s == S - 1 and g == NGRP - 1:
                        rings_o = [nc.gpsimd, nc.scalar, nc.gpsimd, nc.sync]
                        for j in range(PTG):
                            for ki in range(len(klist)):
                                mm(ps[j], g * PTG + j, ki)
                            epi(ps[j], g * PTG + j, rings_o[j % 4])
                    else:
                        for ki in range(len(klist)):
                            for j in range(PTG):
                                mm(ps[j], g * PTG + j, ki)
                        for j in range(PTG):
                            epi(ps[j], g * PTG + j, nc.gpsimd)
    nc.finalize()
    return nc


def prep_inputs(features, weights, bias, class_id):
    f = np.asarray(features, dtype=np.float32)
    w = np.asarray(weights, dtype=np.float32)
    b = np.asarray(bias, dtype=np.float32)
    cid = np.asarray(class_id).astype(np.int64)

    pad = np.zeros((B, CIN, HR, W + 2), np.float32)
    pad[:, :, 1 : H + 1, 1 : W + 1] = f
    xs = np.stack([pad[:, :, :, k : k + W] for k in range(KK)],
                  axis=1).reshape(B, KK, CIN, FL)
    xhi = xs.astype(NP8)
    xlo = (xs - xhi.astype(np.float32)).astype(NP8)

    xa = np.zeros((B, 128, 2, NSUB, FL), NP8)
    for h, src in enumerate((xhi, xlo)):
        for ci in range(2):
            for kw in range(KK):
                xa[:, :, h, ci * KK + kw] = src[:, kw,
                                                ci * 128 : (ci + 1) * 128]
        xa[:, 0:64, h, 6] = src[:, 0, 256:320]
        xa[:, 64:128, h, 6] = src[:, 1, 256:320]
        xa[:, 0:64, h, 7] = src[:, 2, 256:320]
        xa[:, 64:128, h, 7, : FL - W] = src[:, 2, 256:320, W:]

    wsel = w[cid]                                   # [B, COUT, CIN, 3, 3]
    wtb = np.ascontiguousarray(
        wsel.transpose(0, 3, 4, 2, 1).reshape(B, KK * KK, CIN, COUT))
    wsc = wtb * WSCALE
    whi_8 = wsc.astype(NP8)
    wlo_8 = (wsc - whi_8.astype(np.float32)).astype(NP8)

    def t(kh, kw):
        return kh * KK + kw

    # per-instruction weight blocks [B, 128, NWB, 2, COUT]
    w8 = np.zeros((B, 128, NWB, 2, COUT), NP8)
    for half, wsrc in enumerate((whi_8, wlo_8)):
        for ro in range(KK):
            for m in range(NPAIR):
                ii = (half * KK + ro) * NPAIR + m
                if m == 0:
                    w8[:, :, ii, 0] = wsrc[:, t(ro, 0), 0:128]
                    w8[:, :, ii, 1] = wsrc[:, t(ro, 1), 0:128]
                elif m == 1:
                    w8[:, :, ii, 0] = wsrc[:, t(ro, 2), 0:128]
                    w8[:, :, ii, 1] = wsrc[:, t(ro, 0), 128:256]
                elif m == 2:
                    w8[:, :, ii, 0] = wsrc[:, t(ro, 1), 128:256]
                    w8[:, :, ii, 1] = wsrc[:, t(ro, 2), 128:256]
                else:
                    w8[:, 0:64, ii, 0] = wsrc[:, t(ro, 0), 256:320]
                    w8[:, 64:128, ii, 0] = wsrc[:, t(ro, 1), 256:320]
                    if ro == 0:
                        w8[:, 0:64, ii, 1] = wsrc[:, t(0, 2), 256:320]
                        w8[:, 64:128, ii, 1] = wsrc[:, t(1, 2), 256:320]
                    elif ro == 1:
                        w8[:, 64:128, ii, 1] = wsrc[:, t(2, 2), 256:320]
    w8 = w8.reshape(B, 128, NWB * 2 * COUT)

    bsel = np.ascontiguousarray(
        np.broadcast_to(WSCALE * b[cid][:, None, :], (B, 128, COUT)))

    in_maps = []
    for core in range(NCORES):
        sl = slice(core * S, (core + 1) * S)
        in_maps.append({
            "xall": np.ascontiguousarray(xa[sl]),
            "w8d": np.ascontiguousarray(w8[sl]),
            "biasb": np.ascontiguousarray(bsel[sl]),
        })
    return in_maps


def run(features, weights, bias, class_id, trace=False):
    in_maps = prep_inputs(features, weights, bias, class_id)
    nc = build_nc()
    last_exc = None
    for attempt in range(2):
        try:
            res = run_bass_kernel_spmd(nc, in_maps,
                                       core_ids=list(range(NCORES)),
                                       trace=trace)
            break
        except Exception as exc:
            last_exc = exc
            time.sleep(10)
    else:
        raise last_exc
    out = np.concatenate(
        [r["y"].reshape(S, NPIX, COUT).transpose(0, 2, 1)
          .reshape(S, COUT, H, W)
         for r in res.results], axis=0)
    return np.ascontiguousarray(out / WSCALE), res


def kernel(features, weights, bias, class_id):
    out, _ = run(features, weights, bias, class_id)
    return out


# revision 23
# speedup vs baseline: 1.7991x; 1.0024x over previous
"""MoE per-sample expert conv3x3 on 8 trn2 cores — fp8 DoubleRow variant.

Same transposed formulation as kernel.py (pixels = PE partitions, COUT=320
= moving free dim), but the contraction runs in fp8-e4m3 DoubleRow mode:
each matmul contracts TWO 128-row planes (plane stride = one sub-image) at
0.5 cycles per output column — 4x the bf16 row throughput.

Precision: residual split x = x_hi + x_lo, w = w_hi + w_lo (all e4m3, w
pre-scaled by 64 so w_hi/w_lo stay in e4m3's normal range; the 64 comes
back out on host). Three passes accumulate x_hi*w_hi + x_lo*w_hi +
x_hi*w_lo in fp32 PSUM; the dropped x_lo*w_lo term is O(eps^2) ~ 2e-3
relative. Measured end-to-end error ~4e-3 vs the 2e-2 gate.

Layout: 16 sub-images (8 hi + 8 lo) of 66 rows x 64 cols per sample in one
SBUF megatile:
  s0-s2: CIN 0:128   @ kw=0,1,2     s3-s5: CIN 128:256 @ kw=0,1,2
  s6:    [CIN 256:320 @ kw0 | @ kw1]
  s7:    [CIN 256:320 @ kw2 | @ kw2 shifted one image row]
DoubleRow pairs are adjacent sub-images (0,1) (2,3) (4,5) (6,7) at plane
stride FL; the window row offset ro=kh is shared by both planes. Per pass
and ro that is 4 instructions; 3 passes x 3 ro x 4 = 36 per pixel-tile
(5760 PE cycles vs 7360 for the bf16 kernel). Tap/channel assignment,
including ZERO weights for duplicate or invalid plane slots (C2 covers
(0,2)+(1,2) at ro=0 and (2,2) at ro=1-upper; everything else zero), is
host-packed into a per-instruction [128, 2, 320] fp8 weight block — the
device issues 36 uniform matmuls.

Everything else (schedule, rings, epilogue, warmup) matches kernel.py.
"""

import time

import numpy as np
import ml_dtypes

import concourse.bass as bass
import concourse.mybir as mybir
import concourse.tile as tile
from concourse import bacc
from concourse.bass_utils import run_bass_kernel_spmd

B = 16
NCORES = 8
S = B // NCORES
CIN = 320
COUT = 320
H = W = 64
KK = 3
HR = H + 2
NPIX = H * W
FL = HR * W              # 4224
NSUB = 8                 # sub-images per precision half
NPASS = 3                # hh, lh, hl (hh and lh share weight blocks)
NPAIR = 4
NI = NPASS * KK * NPAIR  # 36 DoubleRow instructions per pixel-tile
NWB = 2 * KK * NPAIR     # 24 stored weight blocks (w_hi + w_lo sets)
PTG = 4
NGRP = (NPIX // 128) // PTG
XP = [(0, 10), (10, 26), (26, 42), (42, 58), (58, 66)]

DT8 = mybir.dt.float8e4
NP8 = ml_dtypes.float8_e4m3
WSCALE = 64.0
WARMUP = 7


def build_nc():
    nc = bacc.Bacc(None, target_bir_lowering=False)
    # [S, 128, hi/lo, 8 sub-images, FL]
    xall = nc.dram_tensor("xall", [S, 128, 2, NSUB, FL], DT8,
                          kind="ExternalInput")
    # [S, 128, 24 weight blocks x 2 planes x COUT]
    w8d = nc.dram_tensor("w8d", [S, 128, NWB * 2 * COUT], DT8,
                         kind="ExternalInput")
    biasb = nc.dram_tensor("biasb", [S, 128, COUT], mybir.dt.float32,
                           kind="ExternalInput")
    y = nc.dram_tensor("y", [S, NPIX, COUT], mybir.dt.float32,
                       kind="ExternalOutput")

    with tile.TileContext(nc) as tc:
        with (
            tc.tile_pool(name="wpool", bufs=1) as wpool,
            tc.tile_pool(name="xpool", bufs=2) as xpool,
            tc.tile_pool(name="bpool", bufs=2) as bpool,
            tc.tile_pool(name="opool", bufs=8) as opool,
            tc.tile_pool(name="psum", bufs=2, space="PSUM") as psum_pool,
        ):
            warm = wpool.tile([1, 512], mybir.dt.bfloat16, name="warm",
                              tag="warm")
            nc.vector.memset(warm[:, :], 0.0)
            wpsum = psum_pool.tile([128, 512], mybir.dt.float32,
                                   name="wp", tag="p0")
            for _ in range(WARMUP):
                nc.tensor.matmul(wpsum[0:1, :], warm[0:1, 0:1], warm[0:1, :],
                                 start=True, stop=True)

            rings = [nc.sync, nc.scalar]
            rk = [0]

            def dma(out_ap, in_ap):
                rings[rk[0] % 2].dma_start(out=out_ap, in_=in_ap)
                rk[0] += 1

            xbs_all = {}
            wts_all = {}
            for s in range(S):
                xb = xpool.tile([128, 2 * NSUB * FL], DT8, name=f"x_{s}",
                                tag="xb")
                xbv = xb.rearrange("p (h t f) -> p h t f", h=2, f=FL)
                w8 = wpool.tile([128, NWB * 2 * COUT], DT8, name=f"w_{s}",
                                tag=f"w_{s}")

                def xpiece(t0, t1, p, halves=(0, 1)):
                    a, b = XP[p]
                    for h in halves:
                        dma(xbv[:, h, t0:t1, a * W : b * W],
                            xall[s][:, h, t0:t1, a * W : b * W])

                def wchunk(half, ro):
                    # one ro-row of 4 blocks from the hi (0) or lo (1) set
                    c = NPAIR * 2 * COUT
                    o = (half * KK + ro) * c
                    dma(w8[:, o : o + c], w8d[s][:, o : o + c])

                # demand order: hh pass (hi x + hi w), lh (lo x, same w),
                # hl (hi x again, lo w)
                xpiece(0, 3, 0, (0,))
                wchunk(0, 0)
                xpiece(3, 6, 0, (0,))
                wchunk(0, 1)
                xpiece(6, 8, 0, (0,))
                wchunk(0, 2)
                xpiece(0, 3, 0, (1,))
                xpiece(3, 6, 0, (1,))
                xpiece(6, 8, 0, (1,))
                wchunk(1, 0)
                wchunk(1, 1)
                wchunk(1, 2)
                for p in range(1, len(XP)):
                    xpiece(0, 3, p)
                    xpiece(3, 6, p)
                    xpiece(6, 8, p)

                xbs_all[s] = xb
                wts_all[s] = w8

            bts = []
            for s in range(S):
                bt = bpool.tile([128, COUT], mybir.dt.float32,
                                name=f"bias{s}", tag="bias")
                nc.gpsimd.dma_start(out=bt[:, :], in_=biasb[s])
                bts.append(bt)

            for s in range(S):
                xbv = xbs_all[s].rearrange("p (t f) -> p t f", f=FL)
                w8v = wts_all[s].rearrange("p (i two o) -> p i two o",
                                           two=2, o=COUT)

                # (x half: 0=hi, 1=lo) per pass: hh, lh, hl
                XHALF = (0, 1, 0)
                klist = [(p, ro, m) for p in range(NPASS)
                         for ro in range(KK) for m in range(NPAIR)]
                klast = len(klist) - 1

                WHALF = (0, 0, 1)   # hh and lh share the w_hi blocks

                def mm(ps_j, pt, ki, c0=0, c1=COUT):
                    p, ro, m = klist[ki]
                    tb = XHALF[p] * NSUB + 2 * m
                    o = (pt * 2 + ro) * W
                    lhsT = xbv[:, tb : tb + 2, o : o + 128]
                    rhs = w8v[:, (WHALF[p] * KK + ro) * NPAIR + m, :, c0:c1]
                    nc.tensor.matmul(ps_j[:, : c1 - c0], lhsT, rhs,
                                     start=(ki == 0), stop=(ki == klast),
                                     perf_mode=mybir.MatmulPerfMode.DoubleRow)

                def epi(ps_j, pt, ring):
                    ot = opool.tile([128, COUT], mybir.dt.float32,
                                    name="ot", tag="ot")
                    nc.vector.tensor_tensor(
                        out=ot[:, :], in0=ps_j[:, :COUT], in1=bts[s][:, :],
                        op=mybir.AluOpType.add)
                    ring.dma_start(out=y[s, pt * 128 : (pt + 1) * 128, :],
                                   in_=ot[:, :])

                for g in range(NGRP):
                    ps = [psum_pool.tile([128, 512], mybir.dt.float32,
                                         name=f"ps{j}", tag=f"p{j}")
                          for j in range(PTG)]
                    if s == S - 1 and g == NGRP - 1:
                        rings_o = [nc.gpsimd, nc.scalar, nc.gpsimd]
                        for j in range(PTG - 1):
                            for ki in range(len(klist)):
                                mm(ps[j], g * PTG + j, ki)
                            epi(ps[j], g * PTG + j, rings_o[j])
                        # very last tile: two column halves in SEPARATE psum
                        # banks (second p0 generation is long free) so the
                        # first half's epilogue overlaps the second half's
                        # matmuls and the drain tail is half a tile deep
                        pt = g * PTG + PTG - 1
                        hc = COUT // 2
                        psB = psum_pool.tile([128, 512], mybir.dt.float32,
                                             name="psB", tag="p0")
                        ot = opool.tile([128, COUT], mybir.dt.float32,
                                        name="ot", tag="ot")
                        for (a, c), pj, ring in ((( 0, hc), ps[3], nc.gpsimd),
                                                 ((hc, COUT), psB, nc.sync)):
                            for ki in range(len(klist)):
                                mm(pj, pt, ki, a, c)
                            nc.vector.tensor_tensor(
                                out=ot[:, a:c], in0=pj[:, : c - a],
                                in1=bts[s][:, a:c], op=mybir.AluOpType.add)
                            ring.dma_start(
                                out=y[s, pt * 128 : (pt + 1) * 128, a:c],
                                in_=ot[:, a:c])
                    else:
                        for ki in range(len(klist)):
                            for j in range(PTG):
                                mm(ps[j], g * PTG + j, ki)
                        for j in range(PTG):
                            epi(ps[j], g * PTG + j, nc.gpsimd)
    nc.finalize()
    return nc


def prep_inputs(features, weights, bias, class_id):
    f = np.asarray(features, dtype=np.float32)
    w = np.asarray(weights, dtype=np.float32)
    b = np.asarray(bias, dtype=np.float32)
    cid = np.asarray(class_id).astype(np.int64)

    pad = np.zeros((B, CIN, HR, W + 2), np.float32)
    pad[:, :, 1 : H + 1, 1 : W + 1] = f
    xs = np.stack([pad[:, :, :, k : k + W] for k in range(KK)],
                  axis=1).reshape(B, KK, CIN, FL)
    xhi = xs.astype(NP8)
    xlo = (xs - xhi.astype(np.float32)).astype(NP8)

    xa = np.zeros((B, 128, 2, NSUB, FL), NP8)
    for h, src in enumerate((xhi, xlo)):
        for ci in range(2):
            for kw in range(KK):
                xa[:, :, h, ci * KK + kw] = src[:, kw,
                                                ci * 128 : (ci + 1) * 128]
        xa[:, 0:64, h, 6] = src[:, 0, 256:320]
        xa[:, 64:128, h, 6] = src[:, 1, 256:320]
        xa[:, 0:64, h, 7] = src[:, 2, 256:320]
        xa[:, 64:128, h, 7, : FL - W] = src[:, 2, 256:320, W:]

    wsel = w[cid]                                   # [B, COUT, CIN, 3, 3]
    wtb = np.ascontiguousarray(
        wsel.transpose(0, 3, 4, 2, 1).reshape(B, KK * KK, CIN, COUT))
    wsc = wtb * WSCALE
    whi_8 = wsc.astype(NP8)
    wlo_8 = (wsc - whi_8.astype(np.float32)).astype(NP8)

    def t(kh, kw):
        return kh * KK + kw

    # per-instruction weight blocks [B, 128, NWB, 2, COUT]
    w8 = np.zeros((B, 128, NWB, 2, COUT), NP8)
    for half, wsrc in enumerate((whi_8, wlo_8)):
        for ro in range(KK):
            for m in range(NPAIR):
                ii = (half * KK + ro) * NPAIR + m
                if m == 0:
                    w8[:, :, ii, 0] = wsrc[:, t(ro, 0), 0:128]
                    w8[:, :, ii, 1] = wsrc[:, t(ro, 1), 0:128]
                elif m == 1:
                    w8[:, :, ii, 0] = wsrc[:, t(ro, 2), 0:128]
                    w8[:, :, ii, 1] = wsrc[:, t(ro, 0), 128:256]
                elif m == 2:
                    w8[:, :, ii, 0] = wsrc[:, t(ro, 1), 128:256]
                    w8[:, :, ii, 1] = wsrc[:, t(ro, 2), 128:256]
                else:
                    w8[:, 0:64, ii, 0] = wsrc[:, t(ro, 0), 256:320]
                    w8[:, 64:128, ii, 0] = wsrc[:, t(ro, 1), 256:320]
                    if ro == 0:
                        w8[:, 0:64, ii, 1] = wsrc[:, t(0, 2), 256:320]
                        w8[:, 64:128, ii, 1] = wsrc[:, t(1, 2), 256:320]
                    elif ro == 1:
                        w8[:, 64:128, ii, 1] = wsrc[:, t(2, 2), 256:320]
    w8 = w8.reshape(B, 128, NWB * 2 * COUT)

    bsel = np.ascontiguousarray(
        np.broadcast_to(WSCALE * b[cid][:, None, :], (B, 128, COUT)))

    in_maps = []
    for core in range(NCORES):
        sl = slice(core * S, (core + 1) * S)
        in_maps.append({
            "xall": np.ascontiguousarray(xa[sl]),
            "w8d": np.ascontiguousarray(w8[sl]),
            "biasb": np.ascontiguousarray(bsel[sl]),
        })
    return in_maps


def run(features, weights, bias, class_id, trace=False):
    in_maps = prep_inputs(features, weights, bias, class_id)
    nc = build_nc()
    last_exc = None
    for attempt in range(2):
        try:
            res = run_bass_kernel_spmd(nc, in_maps,
                                       core_ids=list(range(NCORES)),
                                       trace=trace)
            break
        except Exception as exc:
            last_exc = exc
            time.sleep(10)
    else:
        raise last_exc
    out = np.concatenate(
        [r["y"].reshape(S, NPIX, COUT).transpose(0, 2, 1)
          .reshape(S, COUT, H, W)
         for r in res.results], axis=0)
    return np.ascontiguousarray(out / WSCALE), res


def kernel(features, weights, bias, class_id):
    out, _ = run(features, weights, bias, class_id)
    return out
